# revision 5
# baseline (speedup 1.0000x reference)
"""Mixture-of-Experts (E=8, top-2) — expert-parallel Trainium2 Bass kernel.

Strategy (expert-parallel, per the sharding hint):
  * Host computes the router (logits -> top-2 -> softmax) in numpy; this is the
    token->core sharding decision.
  * Tokens are gathered per expert, padded to a fixed capacity C, and core e
    runs expert e's full MLP  y = w2 @ gelu(w1 @ x + b1) + b2  over its tokens
    (bf16 matmuls, fp32 PSUM accumulation, exact-gelu ACT eviction).
  * Host applies the top-2 combine weights and scatter-adds the two expert
    outputs per token (the unshard step).

Device layout (per core, SPMD — same program, per-core data):
  xt  [D, C]  bf16   gathered tokens, transposed (token dim = free dim)
  w1t [D, F]  bf16   w1[e].T     (contraction dim D on partitions)
  w2t [F, D]  bf16   w2[e].T     (contraction dim F on partitions)
  b1r [128, F/128]   b1 as per-partition bias table
  b2r [128, D/128]   b2 as per-partition bias table
  yt  [D, C]  bf16   expert output, transposed

Perf notes (v2, from NTFF trace of v1 @505us):
  * PE stream is the roofline: N=512 chunks cut per-MM NX overhead vs N=272.
  * Startup was 16.3us to first MM: fix with fine-grained early w1 pieces on
    the scalar HWDGE queue, chunk-0 x split per k-tile on sync, and ~20
    warm-up matmuls on scratch SBUF to ride out the HAM cold clock.
  * w2 arrives over gpsimd behind w1; g0/g1 go early on sync/scalar so
    MLP2 of chunk 0 never stalls.
  * y emitted as bf16 (negligible error, halves out-DMA) in 2 pieces/chunk;
    small 128-token tail chunk goes last so the final DMA drains fast.
"""

import numpy as np
from contextlib import ExitStack

from ml_dtypes import bfloat16

import concourse.bacc as bacc
import concourse.tile as tile
import concourse.mybir as mybir
from concourse.bass_utils import run_bass_kernel_spmd

P = 128
D = 1024
F = 4096
E = 8
TOPK = 2
B, S = 4, 2048

NTB = 512          # big chunk size (matmul free dim; PSUM bank = 512 f32)
C_DEFAULT = 2176   # 4*512 + 128 tail

KD = D // P        # 8  k-tiles for MLP1 (contract D)
KF = F // P        # 32 k-tiles for MLP2 (contract F)
MF = F // P        # 32 m-tiles for MLP1 output (F)
MD = D // P        # 8  m-tiles for MLP2 output (D)

N_WARMUP = 20      # scratch matmuls to warm the PE HAM clock during DMA wait

_prog_cache: dict = {}
ACT_FUNC = None  # default: Gelu; sim_check overrides (CoreSim lacks Gelu)
last_results = None  # BassKernelResults of the most recent run (for test harness)
trace_kwargs: dict = {}  # test harness can set e.g. {"trace": True}


def _chunk_plan(C: int):
    """Split C into chunks of <=512 columns, each >=256 (and a multiple of 16)
    so matmuls stay stream-bound, not LDWEIGHTS-bound. Descending sizes: the
    first chunk is largest (relaxes weight-stream deadlines at startup), the
    last is smallest (fast final DMA drain)."""
    assert C % 16 == 0
    if C <= NTB:
        plan = [C]
    else:
        rest = C - NTB
        n = -(-rest // NTB)
        units = rest // 16
        base, extra = divmod(units, n)
        plan = [NTB] + sorted(
            (16 * (base + (1 if i < extra else 0)) for i in range(n)), reverse=True
        )
        assert plan[-1] >= 256, plan
    offs = np.concatenate([[0], np.cumsum(plan)]).astype(int)
    return [(int(offs[i]), int(plan[i])) for i in range(len(plan))]


def _build_program(C: int):
    """Build + compile the SPMD single-expert MLP program for capacity C."""
    bf16 = mybir.dt.bfloat16
    f32 = mybir.dt.float32
    chunks = _chunk_plan(C)

    nc = bacc.Bacc(
        "TRN2",
        target_bir_lowering=False,
        debug=False,
        enable_asserts=False,
        num_devices=E,
    )

    xt = nc.dram_tensor("xt", [D, C], bf16, kind="ExternalInput").ap()
    w1t = nc.dram_tensor("w1t", [D, F], bf16, kind="ExternalInput").ap()
    w2t = nc.dram_tensor("w2t", [F, D], bf16, kind="ExternalInput").ap()
    b1r = nc.dram_tensor("b1r", [P, MF], f32, kind="ExternalInput").ap()
    b2r = nc.dram_tensor("b2r", [P, MD], f32, kind="ExternalInput").ap()
    yt = nc.dram_tensor("yt", [D, C], bf16, kind="ExternalOutput").ap()

    # Partition-tiled DRAM views.
    xt_r = xt.rearrange("(k p) c -> p k c", p=P)   # [128, KD, C]
    w1t_r = w1t.rearrange("(k p) f -> p k f", p=P)  # [128, KD, F]
    w2t_r = w2t.rearrange("(k p) d -> p k d", p=P)  # [128, KF, D]
    yt_r = yt.rearrange("(m p) c -> p m c", p=P)    # [128, MD, C]

    with tile.TileContext(nc) as tc, ExitStack() as ctx:
        wpool = ctx.enter_context(tc.tile_pool(name="wpool", bufs=1))
        xpool = ctx.enter_context(tc.tile_pool(name="xpool", bufs=3))
        hpool = ctx.enter_context(tc.tile_pool(name="hpool", bufs=1))
        ypool = ctx.enter_context(tc.tile_pool(name="ypool", bufs=2))
        pspool = ctx.enter_context(tc.tile_pool(name="pspool", bufs=8, space="PSUM"))

        # --- PE warm-up: scratch matmuls with no DMA dependency. They fill
        # the otherwise-idle PE window while the first weights stream in and
        # push the HAM activity monitor to full clock before real MMs start.
        scr = wpool.tile([P, P], bf16, name="scr")
        nc.vector.memset(scr[:, :], 0.0)
        ps_scr = pspool.tile([P, NTB], f32, name="pt")
        for _ in range(N_WARMUP):
            nc.tensor.matmul(
                ps_scr[:, :P], lhsT=scr[:, :], rhs=scr[:, :], start=True, stop=True
            )

        # --- biases (sync, first: b1 is needed by the first ACT eviction) ---
        b1_sb = wpool.tile([P, MF], f32, name="b1sb")
        nc.sync.dma_start(out=b1_sb[:, :], in_=b1r[:, :])
        b2_sb = wpool.tile([P, MD], f32, name="b2sb")

        # --- weights: single resident tiles, streamed in consumption order ---
        w1_sb = wpool.tile([P, KD, F], bf16, name="w1sb")
        w2_sb = wpool.tile([P, KF, D], bf16, name="w2sb")

        # chunk-0 x, split per k-tile so the first MM group can start as
        # soon as (w1 m0 piece, x k0 piece) land.
        nt0 = chunks[0][1]
        x_first = xpool.tile([P, KD, NTB], bf16, name="xtile")
        for k in range(KD):
            nc.sync.dma_start(out=x_first[:, k, :nt0], in_=xt_r[:, k, 0:nt0])

        # scalar (HWDGE): first 1MB of w1 (m0 in k-halves, then m1-m3), then
        # w2 g1, then it goes quiet until the y-out DMAs. Nothing else early:
        # the bulk weight stream needs the HBM port.
        nc.scalar.dma_start(out=w1_sb[:, 0:4, 0:P], in_=w1t_r[:, 0:4, 0:P])
        nc.scalar.dma_start(out=w1_sb[:, 4:KD, 0:P], in_=w1t_r[:, 4:KD, 0:P])
        for m in (1, 2, 3):
            nc.scalar.dma_start(
                out=w1_sb[:, :, m * P : (m + 1) * P],
                in_=w1t_r[:, :, m * P : (m + 1) * P],
            )
        nc.scalar.dma_start(out=w2_sb[:, 8:16, :], in_=w2t_r[:, 8:16, :])

        # gpsimd (SWDGE, ~275GB/s solo): the bulk stream in consumption
        # order: w1 m4..m31, then w2 g2, g3.
        for f0, f1 in ((512, 1024), (1024, 2048), (2048, 3072), (3072, 4096)):
            nc.gpsimd.dma_start(out=w1_sb[:, :, f0:f1], in_=w1t_r[:, :, f0:f1])
        nc.gpsimd.dma_start(out=w2_sb[:, 16:24, :], in_=w2t_r[:, 16:24, :])
        nc.gpsimd.dma_start(out=w2_sb[:, 24:32, :], in_=w2t_r[:, 24:32, :])

        # sync: w2 g0 after chunk-0 x, then the remaining x chunks + b2.
        nc.sync.dma_start(out=w2_sb[:, 0:8, :], in_=w2t_r[:, 0:8, :])
        x_tiles = [x_first]
        for ci in range(1, len(chunks)):
            off, nt = chunks[ci]
            t = xpool.tile([P, KD, NTB], bf16, name="xtile")
            nc.sync.dma_start(out=t[:, :, :nt], in_=xt_r[:, :, off : off + nt])
            x_tiles.append(t)
        nc.sync.dma_start(out=b2_sb[:, :], in_=b2r[:, :])

        for ci, (off, nt) in enumerate(chunks):
            x_sb = x_tiles[ci]

            # MLP1: h[F, nt] = gelu(w1t.T @ x + b1), bf16 out
            h_sb = hpool.tile([P, KF, NTB], bf16, name="htile")
            for m in range(MF):
                pt = pspool.tile([P, NTB], f32, name="pt")
                for k in range(KD):
                    nc.tensor.matmul(
                        pt[:, :nt],
                        lhsT=w1_sb[:, k, m * P : (m + 1) * P],
                        rhs=x_sb[:, k, :nt],
                        start=(k == 0),
                        stop=(k == KD - 1),
                    )
                nc.scalar.activation(
                    h_sb[:, m, :nt],
                    pt[:, :nt],
                    ACT_FUNC or mybir.ActivationFunctionType.Gelu,
                    bias=b1_sb[:, m : m + 1],
                )

            # MLP2: y[D, nt] = w2t.T @ h + b2, bf16 out
            y_sb = ypool.tile([P, MD, NTB], bf16, name="ytile")
            for m in range(MD):
                pt = pspool.tile([P, NTB], f32, name="pt")
                for k in range(KF):
                    nc.tensor.matmul(
                        pt[:, :nt],
                        lhsT=w2_sb[:, k, m * P : (m + 1) * P],
                        rhs=h_sb[:, k, :nt],
                        start=(k == 0),
                        stop=(k == KF - 1),
                    )
                nc.scalar.activation(
                    y_sb[:, m, :nt],
                    pt[:, :nt],
                    mybir.ActivationFunctionType.Identity,
                    bias=b2_sb[:, m : m + 1],
                )
                if m == MD // 2 - 1 or m == MD - 1:
                    h0 = m + 1 - MD // 2
                    nc.scalar.dma_start(
                        out=yt_r[:, h0 : m + 1, off : off + nt],
                        in_=y_sb[:, h0 : m + 1, :nt],
                    )

    nc.compile()
    return nc


def _get_program(C: int):
    if C not in _prog_cache:
        _prog_cache[C] = _build_program(C)
    return _prog_cache[C]


def _route(xf: np.ndarray, router_w: np.ndarray):
    """Top-2 routing identical to the reference (ties -> lower expert idx).

    Logits in fp64 so the selection is independent of BLAS blocking/threads
    (top-2 gaps in this regime are >= ~3e-6; fp64 noise is ~1e-15).
    """
    logits = xf.astype(np.float64) @ router_w.T.astype(np.float64)  # [T, E]
    idx = np.argsort(-logits, axis=1, kind="stable")[:, :TOPK]
    vals = np.take_along_axis(logits, idx, axis=1)
    vals = vals - vals.max(axis=1, keepdims=True)
    ev = np.exp(vals)
    probs = (ev / ev.sum(axis=1, keepdims=True)).astype(np.float32)
    return idx.astype(np.int64), probs


def kernel(x, router_w, w1, b1, w2, b2):
    global last_results

    x = np.asarray(x, dtype=np.float32)
    router_w = np.asarray(router_w, dtype=np.float32)
    w1 = np.asarray(w1, dtype=np.float32)
    b1 = np.asarray(b1, dtype=np.float32)
    w2 = np.asarray(w2, dtype=np.float32)
    b2 = np.asarray(b2, dtype=np.float32)

    orig_shape = x.shape
    xf = x.reshape(-1, D)
    T = xf.shape[0]

    idx, probs = _route(xf, router_w)

    # Group the (token, k) pairs by expert; rank = position within the expert.
    flat_e = idx.ravel()  # entry j corresponds to token j//2, slot j%2
    order = np.argsort(flat_e, kind="stable")
    counts = np.bincount(flat_e, minlength=E)
    starts = np.zeros(E + 1, dtype=np.int64)
    np.cumsum(counts, out=starts[1:])
    rank = np.empty(2 * T, dtype=np.int64)
    rank[order] = np.arange(2 * T, dtype=np.int64) - starts[flat_e[order]]
    pos = rank.reshape(T, TOPK)

    cmax = int(counts.max())
    C = C_DEFAULT if cmax <= C_DEFAULT else int(-(-cmax // 64) * 64)
    nc = _get_program(C)

    xf_bf = xf.astype(bfloat16)
    in_maps = []
    for e in range(E):
        toks = order[starts[e] : starts[e + 1]] // 2
        xt = np.zeros((D, C), dtype=bfloat16)
        xt[:, : len(toks)] = xf_bf[toks].T
        in_maps.append(
            {
                "xt": xt,
                "w1t": np.ascontiguousarray(w1[e].T).astype(bfloat16),
                "w2t": np.ascontiguousarray(w2[e].T).astype(bfloat16),
                "b1r": np.ascontiguousarray(b1[e].reshape(MF, P).T),
                "b2r": np.ascontiguousarray(b2[e].reshape(MD, P).T),
            }
        )

    res = run_bass_kernel_spmd(nc, in_maps, core_ids=list(range(E)), **trace_kwargs)
    last_results = res

    ys = np.stack([np.asarray(r["yt"]) for r in res.results])  # [E, D, C] bf16
    out = probs[:, 0:1] * ys[idx[:, 0], :, pos[:, 0]].astype(np.float32)
    out += probs[:, 1:2] * ys[idx[:, 1], :, pos[:, 1]].astype(np.float32)
    return out.astype(np.float32).reshape(orig_shape)


# revision 6
# speedup vs baseline: 1.2661x; 1.2661x over previous
"""Mixture-of-Experts (E=8, top-2) — expert-parallel Trainium2 Bass kernel.

Strategy (expert-parallel, per the sharding hint):
  * Host computes the router (logits -> top-2 -> softmax) in numpy; this is the
    token->core sharding decision.
  * Tokens are gathered per expert, padded to a fixed capacity C, and core e
    runs expert e's full MLP  y = w2 @ gelu(w1 @ x + b1) + b2  over its tokens
    (bf16 matmuls, fp32 PSUM accumulation, exact-gelu ACT eviction).
  * Host applies the top-2 combine weights and scatter-adds the two expert
    outputs per token (the unshard step).

Device layout (per core, SPMD — same program, per-core data). All DRAM
tensors are packed host-side so that every DMA piece is CONTIGUOUS per
partition row — DMA queue throughput is set by descriptor size (fragmented
256B-1KB runs collapse HWDGE queues to 15-30GB/s; 2-8KB runs sustain
~275GB/s+):
  xp  [NC, P, KD*NTB] bf16  per-chunk token tiles ([p][k][j], j=token in chunk)
  w1p [MF, P, KD*P]   bf16  w1[e].T as per-m-tile pieces ([m][p][k][j])
  w2p [KF, P, MD*P]   bf16  w2[e].T as per-k-tile pieces ([k][p][m][j])
  b1r [P, MF] f32, b2r [P, MD] f32   per-partition bias tables
  yp  [NC, P, MD*NTB] bf16  per-chunk output tiles ([p][m][j])

Perf notes (vs the 505us v1 baseline):
  * Chunks of 512..256 tokens (all >=256 so matmuls stay stream-bound and
    LDWEIGHTS stays hidden; descending sizes: big first chunk relaxes the
    weight-stream deadlines, small last chunk drains the final DMA fast).
  * Startup: ~20 warm-up matmuls on scratch SBUF ride out the HAM cold clock
    while the first weight/token pieces land; first w1 m-tiles go in fine
    pieces on the scalar HWDGE queue, chunk-0 x per k-tile on sync.
  * Bulk weights stream on gpsimd (SWDGE) in consumption order; w2 g0/g1 go
    early on sync/scalar so MLP2 of chunk 0 never stalls.
  * y emitted as bf16 (negligible error, halves out-DMA).
"""

import numpy as np
from contextlib import ExitStack

from ml_dtypes import bfloat16

import concourse.bacc as bacc
import concourse.tile as tile
import concourse.mybir as mybir
from concourse.bass_utils import run_bass_kernel_spmd

P = 128
D = 1024
F = 4096
E = 8
TOPK = 2
B, S = 4, 2048

NTB = 512          # max chunk size (matmul free dim; PSUM bank = 512 f32)
C_DEFAULT = 2176   # covers cmax for the reference data; 512+4*416

KD = D // P        # 8  k-tiles for MLP1 (contract D)
KF = F // P        # 32 k-tiles for MLP2 (contract F)
MF = F // P        # 32 m-tiles for MLP1 output (F)
MD = D // P        # 8  m-tiles for MLP2 output (D)

N_WARMUP = 20      # scratch matmuls to warm the PE HAM clock during DMA wait

_prog_cache: dict = {}
ACT_FUNC = None  # default: Gelu; sim_check overrides (CoreSim lacks Gelu)
last_results = None  # BassKernelResults of the most recent run (for test harness)
trace_kwargs: dict = {}  # test harness can set e.g. {"trace": True}


def _chunk_plan(C: int):
    """Chunks of <=512 columns, each >=256 (multiples of 16), descending."""
    assert C % 16 == 0
    if C <= NTB:
        plan = [C]
    else:
        rest = C - NTB
        n = -(-rest // NTB)
        units = rest // 16
        base, extra = divmod(units, n)
        plan = [NTB] + sorted(
            (16 * (base + (1 if i < extra else 0)) for i in range(n)), reverse=True
        )
        assert plan[-1] >= 256, plan
    offs = np.concatenate([[0], np.cumsum(plan)]).astype(int)
    return [(int(offs[i]), int(plan[i])) for i in range(len(plan))]


def _build_program(C: int):
    """Build + compile the SPMD single-expert MLP program for capacity C."""
    bf16 = mybir.dt.bfloat16
    f32 = mybir.dt.float32
    chunks = _chunk_plan(C)
    NC = len(chunks)

    nc = bacc.Bacc(
        "TRN2",
        target_bir_lowering=False,
        debug=False,
        enable_asserts=False,
        num_devices=E,
    )

    xp = nc.dram_tensor("xp", [NC, P, KD * NTB], bf16, kind="ExternalInput").ap()
    w1p = nc.dram_tensor("w1p", [MF, P, KD * P], bf16, kind="ExternalInput").ap()
    w2p = nc.dram_tensor("w2p", [KF, P, MD * P], bf16, kind="ExternalInput").ap()
    b1r = nc.dram_tensor("b1r", [P, MF], f32, kind="ExternalInput").ap()
    b2r = nc.dram_tensor("b2r", [P, MD], f32, kind="ExternalInput").ap()
    yp = nc.dram_tensor("yp", [NC, P, MD * NTB], bf16, kind="ExternalOutput").ap()

    # Partition-first views (contiguous innermost per partition row).
    xp_r = xp.rearrange("c p x -> p c x")    # [P, NC, KD*NTB]
    w1p_r = w1p.rearrange("m p x -> p m x")  # [P, MF, KD*P]
    w2p_r = w2p.rearrange("k p x -> p k x")  # [P, KF, MD*P]
    yp_r = yp.rearrange("c p x -> p c x")    # [P, NC, MD*NTB]

    with tile.TileContext(nc) as tc, ExitStack() as ctx:
        wpool = ctx.enter_context(tc.tile_pool(name="wpool", bufs=1))
        xpool = ctx.enter_context(tc.tile_pool(name="xpool", bufs=3))
        hpool = ctx.enter_context(tc.tile_pool(name="hpool", bufs=1))
        ypool = ctx.enter_context(tc.tile_pool(name="ypool", bufs=2))
        pspool = ctx.enter_context(tc.tile_pool(name="pspool", bufs=8, space="PSUM"))

        # --- PE warm-up: scratch matmuls with no DMA dependency. They fill
        # the otherwise-idle PE window while the first weights stream in and
        # push the HAM activity monitor to full clock before real MMs start.
        scr = wpool.tile([P, P], bf16, name="scr")
        nc.vector.memset(scr[:, :], 0.0)
        ps_scr = pspool.tile([P, NTB], f32, name="pt")
        for _ in range(N_WARMUP):
            nc.tensor.matmul(
                ps_scr[:, :P], lhsT=scr[:, :], rhs=scr[:, :], start=True, stop=True
            )

        # --- biases (sync, first: b1 is needed by the first ACT eviction) ---
        b1_sb = wpool.tile([P, MF], f32, name="b1sb")
        nc.sync.dma_start(out=b1_sb[:, :], in_=b1r[:, :])
        b2_sb = wpool.tile([P, MD], f32, name="b2sb")

        # --- weights: single resident tiles (m-/k-piece-contiguous layout),
        # streamed in consumption order.
        w1_sb = wpool.tile([P, MF, KD, P], bf16, name="w1sb")
        w2_sb = wpool.tile([P, KF, MD, P], bf16, name="w2sb")

        # chunk-0 x per k-tile so the first MM group starts as soon as the
        # (w1 m0, x k0) pieces land.
        x_first = xpool.tile([P, KD, NTB], bf16, name="xtile")
        for k in range(KD):
            nc.sync.dma_start(
                out=x_first[:, k, :], in_=xp_r[:, 0, k * NTB : (k + 1) * NTB]
            )

        # scalar (HWDGE): first w1 m-tiles in fine pieces, then w2 g1, then
        # quiet until the y-out DMAs.
        for m in range(4):
            nc.scalar.dma_start(
                out=w1_sb[:, m, :, :], in_=w1p_r[:, m, :]
            )
        nc.scalar.dma_start(out=w2_sb[:, 8:16, :, :], in_=w2p_r[:, 8:16, :])

        # gpsimd (SWDGE): bulk stream in consumption order: w1 m4..m31,
        # then w2 g2, g3.
        for m0, m1 in ((4, 8), (8, 16), (16, 24), (24, 32)):
            nc.gpsimd.dma_start(out=w1_sb[:, m0:m1, :, :], in_=w1p_r[:, m0:m1, :])
        nc.gpsimd.dma_start(out=w2_sb[:, 16:24, :, :], in_=w2p_r[:, 16:24, :])
        nc.gpsimd.dma_start(out=w2_sb[:, 24:32, :, :], in_=w2p_r[:, 24:32, :])

        # sync: w2 g0 after chunk-0 x, then the remaining x chunks + b2.
        nc.sync.dma_start(out=w2_sb[:, 0:8, :, :], in_=w2p_r[:, 0:8, :])
        x_tiles = [x_first]
        for ci in range(1, NC):
            t = xpool.tile([P, KD, NTB], bf16, name="xtile")
            nc.sync.dma_start(out=t[:, :, :], in_=xp_r[:, ci, :])
            x_tiles.append(t)
        nc.sync.dma_start(out=b2_sb[:, :], in_=b2r[:, :])

        for ci, (off, nt) in enumerate(chunks):
            x_sb = x_tiles[ci]

            # MLP1: h[F, nt] = gelu(w1t.T @ x + b1), bf16 out
            h_sb = hpool.tile([P, KF, NTB], bf16, name="htile")
            for m in range(MF):
                pt = pspool.tile([P, NTB], f32, name="pt")
                for k in range(KD):
                    nc.tensor.matmul(
                        pt[:, :nt],
                        lhsT=w1_sb[:, m, k, :],
                        rhs=x_sb[:, k, :nt],
                        start=(k == 0),
                        stop=(k == KD - 1),
                    )
                nc.scalar.activation(
                    h_sb[:, m, :nt],
                    pt[:, :nt],
                    ACT_FUNC or mybir.ActivationFunctionType.Gelu,
                    bias=b1_sb[:, m : m + 1],
                )

            # MLP2: y[D, nt] = w2t.T @ h + b2, bf16 out
            y_sb = ypool.tile([P, MD, NTB], bf16, name="ytile")
            for m in range(MD):
                pt = pspool.tile([P, NTB], f32, name="pt")
                for k in range(KF):
                    nc.tensor.matmul(
                        pt[:, :nt],
                        lhsT=w2_sb[:, k, m, :],
                        rhs=h_sb[:, k, :nt],
                        start=(k == 0),
                        stop=(k == KF - 1),
                    )
                nc.scalar.activation(
                    y_sb[:, m, :nt],
                    pt[:, :nt],
                    mybir.ActivationFunctionType.Identity,
                    bias=b2_sb[:, m : m + 1],
                )
                if m == MD // 2 - 1 or m == MD - 1:
                    h0 = m + 1 - MD // 2
                    nc.scalar.dma_start(
                        out=yp_r[:, ci, h0 * NTB : (m + 1) * NTB],
                        in_=y_sb[:, h0 : m + 1, :],
                    )

    nc.compile()
    return nc


def _get_program(C: int):
    if C not in _prog_cache:
        _prog_cache[C] = _build_program(C)
    return _prog_cache[C]


def _route(xf: np.ndarray, router_w: np.ndarray):
    """Top-2 routing identical to the reference (ties -> lower expert idx).

    Logits in fp64 so the selection is independent of BLAS blocking/threads
    (top-2 gaps in this regime are >= ~3e-6; fp64 noise is ~1e-15).
    """
    logits = xf.astype(np.float64) @ router_w.T.astype(np.float64)  # [T, E]
    idx = np.argsort(-logits, axis=1, kind="stable")[:, :TOPK]
    vals = np.take_along_axis(logits, idx, axis=1)
    vals = vals - vals.max(axis=1, keepdims=True)
    ev = np.exp(vals)
    probs = (ev / ev.sum(axis=1, keepdims=True)).astype(np.float32)
    return idx.astype(np.int64), probs


def kernel(x, router_w, w1, b1, w2, b2):
    global last_results

    x = np.asarray(x, dtype=np.float32)
    router_w = np.asarray(router_w, dtype=np.float32)
    w1 = np.asarray(w1, dtype=np.float32)
    b1 = np.asarray(b1, dtype=np.float32)
    w2 = np.asarray(w2, dtype=np.float32)
    b2 = np.asarray(b2, dtype=np.float32)

    orig_shape = x.shape
    xf = x.reshape(-1, D)
    T = xf.shape[0]

    idx, probs = _route(xf, router_w)

    # Group the (token, k) pairs by expert; rank = position within the expert.
    flat_e = idx.ravel()  # entry j corresponds to token j//2, slot j%2
    order = np.argsort(flat_e, kind="stable")
    counts = np.bincount(flat_e, minlength=E)
    starts = np.zeros(E + 1, dtype=np.int64)
    np.cumsum(counts, out=starts[1:])
    rank = np.empty(2 * T, dtype=np.int64)
    rank[order] = np.arange(2 * T, dtype=np.int64) - starts[flat_e[order]]
    pos = rank.reshape(T, TOPK)

    cmax = int(counts.max())
    C = C_DEFAULT if cmax <= C_DEFAULT else int(-(-cmax // 64) * 64)
    nc = _get_program(C)
    chunks = _chunk_plan(C)
    NC = len(chunks)

    xf_bf = xf.astype(bfloat16)
    in_maps = []
    for e in range(E):
        toks = order[starts[e] : starts[e + 1]] // 2
        xt = np.zeros((D, C), dtype=bfloat16)
        xt[:, : len(toks)] = xf_bf[toks].T
        # xp[c, p, k*NTB + j] = xt[k*128+p, off_c + j]
        xp = np.zeros((NC, P, KD * NTB), dtype=bfloat16)
        xt4 = xt.reshape(KD, P, C)
        for ci, (off, nt) in enumerate(chunks):
            xp[ci].reshape(P, KD, NTB)[:, :, :nt] = xt4[:, :, off : off + nt].transpose(
                1, 0, 2
            )
        # w1p[m, p, k*128+j] = w1[e][m*128+j, k*128+p]
        w1p = np.ascontiguousarray(
            w1[e].astype(bfloat16).reshape(MF, P, KD, P).transpose(0, 3, 2, 1)
        ).reshape(MF, P, KD * P)
        # w2p[k, p, m*128+j] = w2[e][m*128+j, k*128+p]
        w2p = np.ascontiguousarray(
            w2[e].astype(bfloat16).reshape(MD, P, KF, P).transpose(2, 3, 0, 1)
        ).reshape(KF, P, MD * P)
        in_maps.append(
            {
                "xp": xp,
                "w1p": w1p,
                "w2p": w2p,
                "b1r": np.ascontiguousarray(b1[e].reshape(MF, P).T),
                "b2r": np.ascontiguousarray(b2[e].reshape(MD, P).T),
            }
        )

    res = run_bass_kernel_spmd(nc, in_maps, core_ids=list(range(E)), **trace_kwargs)
    last_results = res

    # Unpack yp [NC, P, MD*NTB] -> ys [E, D, C] f32.
    ys = np.empty((E, D, C), dtype=np.float32)
    for e in range(E):
        ype = np.asarray(res.results[e]["yp"]).reshape(NC, P, MD, NTB)
        for ci, (off, nt) in enumerate(chunks):
            # y[d=m*128+p, off+j] = ype[ci, p, m, j]
            ys[e, :, off : off + nt] = (
                ype[ci, :, :, :nt].transpose(1, 0, 2).reshape(D, nt).astype(np.float32)
            )

    out = probs[:, 0:1] * ys[idx[:, 0], :, pos[:, 0]]
    out += probs[:, 1:2] * ys[idx[:, 1], :, pos[:, 1]]
    return out.astype(np.float32).reshape(orig_shape)
